# revision 21
# baseline (speedup 1.0000x reference)
"""BernNet GNN message-passing kernel for 8 Trainium2 NeuronCores.

Math: reference computes out = sum_m C(K,m)/2^K * relu(temp)[m] * L^m M^{K-m} x
with L = I - Ahat, M = I + Ahat (Ahat = D^-1/2 A D^-1/2) and x = MLP(node_feat).
L and M commute, so out = p(Ahat) x for a degree-K polynomial p whose monomial
coefficients c_j are an exact (host-side, fp64) linear function of relu(temp).
That needs K=10 sparse aggregations instead of the reference's 65.

Sharding: nodes are permuted (per-core contiguous blocks of 12544 = 98*128,
degree-sorted within a core so per-chunk slot padding is tight). Each core owns
the destination rows of its block and the edges into them. Iteration state
z_j = dsq * Ahat^j x is replicated via an AllGather each iteration; per-core
work is an indirect-DMA row gather (256B rows of z) + strided DVE reduction
per 128-destination chunk, then cheap per-row scalings.
"""

import math

import numpy as np

import concourse.bass as bass
import concourse.mybir as mybir
import concourse.tile as tile
from concourse import bacc
from concourse import bass_utils

# Problem constants (hardcoded per contract; kernel.py must be self-contained)
N = 100000
E = 3200000
K = 10
D_IN = 512
D_H = 256
F = 64

NC = 8          # cores
P = 128         # partitions
NPC_REAL = N // NC          # 12500 real nodes per core
NCHUNK = (NPC_REAL + P - 1) // P   # 98
NPC = NCHUNK * P            # 12544 padded nodes per core
# Each core's z shard carries NPC rows + 1 zero row (for padding slots), so the
# AllGather output is the whole gather table and has a single writer.
SHARD = NPC + 1
ZROWS = NC * SHARD          # gather-table rows
ZPAD = NPC                  # index of core 0's zero row (used for all pads)


def _set_problem(n, e):
    """Recompute derived sizes (used by the small-scale sim tests only)."""
    global N, E, NPC_REAL, NCHUNK, NPC, SHARD, ZROWS, ZPAD
    N, E = n, e
    NPC_REAL = N // NC
    NCHUNK = (NPC_REAL + P - 1) // P
    NPC = NCHUNK * P
    SHARD = NPC + 1
    ZROWS = NC * SHARD
    ZPAD = NPC

F32 = mybir.dt.float32
I32 = mybir.dt.int32


def _poly_coeffs(temp: np.ndarray) -> np.ndarray:
    """Monomial coefficients c_j of p(t) = sum_m C(K,m)/2^K relu(temp)[m] (1-t)^m (1+t)^(K-m)."""
    T = np.maximum(temp.astype(np.float64), 0.0)
    c = np.zeros(K + 1, dtype=np.float64)
    for m in range(K + 1):
        a = np.array([1.0])
        for _ in range(m):
            a = np.convolve(a, [1.0, -1.0])   # * (1 - t)
        for _ in range(K - m):
            a = np.convolve(a, [1.0, 1.0])    # * (1 + t)
        c += (math.comb(K, m) / float(2 ** K)) * T[m] * a
    return c


def _host_prep(node_feat, edge_index, temp):
    """Permutation, CSR slot structure, and per-core input shards."""
    row = np.asarray(edge_index[0], dtype=np.int64)
    col = np.asarray(edge_index[1], dtype=np.int64)
    deg = np.bincount(row, minlength=N).astype(np.int64)

    # pi: node -> global padded position. Core c owns originals [c*12500,(c+1)*12500),
    # sorted ascending by degree within the core; pads sit at the low ranks.
    pos = np.empty(N, dtype=np.int64)
    npad = NPC - NPC_REAL
    for c in range(NC):
        ids = np.arange(c * NPC_REAL, (c + 1) * NPC_REAL)
        order = np.argsort(deg[ids], kind="stable")
        pos[ids[order]] = c * NPC + npad + np.arange(NPC_REAL)

    pd = pos[row]
    ps = pos[col]
    order = np.argsort(pd, kind="stable")
    pd_s = pd[order]
    ps_s = ps[order]
    cnt = np.bincount(pd_s, minlength=NC * NPC).astype(np.int64)
    rowptr = np.concatenate([[0], np.cumsum(cnt)])
    slot = np.arange(E, dtype=np.int64) - rowptr[pd_s]

    c_e = pd_s // NPC
    r_e = pd_s % NPC
    k_e = r_e // P
    p_e = r_e % P

    # shared-across-cores slot counts per chunk
    S_arr = np.zeros((NC, NCHUNK), dtype=np.int64)
    np.maximum.at(S_arr, (c_e, k_e), slot + 1)
    S_k = np.maximum(S_arr.max(axis=0), 1).astype(np.int64)
    off = np.concatenate([[0], np.cumsum(S_k)])
    total_S = int(off[-1])

    # table row of pi-position (c, r) is c*SHARD + r (shards carry a zero row)
    ps_row = (ps_s // NPC) * SHARD + (ps_s % NPC)
    idx_all = np.full((NC, P, total_S), ZPAD, dtype=np.int32)
    idx_all[c_e, p_e, off[k_e] + slot] = ps_row.astype(np.int32)

    degpk = cnt.reshape(NC, NCHUNK, P).transpose(0, 2, 1).astype(np.float32)
    degpk = np.ascontiguousarray(degpk)

    nfT = np.zeros((NC, D_IN, NPC), dtype=np.float32)
    cc = pos // NPC
    rr = pos % NPC
    nfT[cc, :, rr] = np.asarray(node_feat, dtype=np.float32)

    cj = _poly_coeffs(np.asarray(temp))
    return dict(
        pos=pos, S_k=S_k, off=off, total_S=total_S,
        idx_all=idx_all, degpk=degpk, nfT=nfT, cj=cj,
    )


def _build_nc(S_k, off, total_S, cj):
    """Build the Bass module (shared across all 8 cores)."""
    nc = bacc.Bacc("TRN2", target_bir_lowering=False, debug=False, num_devices=NC)

    nfT_d = nc.dram_tensor("nfT", [D_IN, NPC], F32, kind="ExternalInput")
    idx_d = nc.dram_tensor("idx", [P, total_S], I32, kind="ExternalInput")
    deg_d = nc.dram_tensor("degpk", [P, NCHUNK], F32, kind="ExternalInput")
    W1_d = nc.dram_tensor("W1", [D_IN, D_H], F32, kind="ExternalInput")
    b1_d = nc.dram_tensor("b1", [D_H], F32, kind="ExternalInput")
    W2_d = nc.dram_tensor("W2", [D_H, F], F32, kind="ExternalInput")
    b2_d = nc.dram_tensor("b2", [F], F32, kind="ExternalInput")
    out_d = nc.dram_tensor("out", [NPC, F], F32, kind="ExternalOutput")

    from concourse.masks import make_identity

    with tile.TileContext(nc) as tc:
        with (
            tc.tile_pool(name="consts", bufs=1) as consts,
            tc.tile_pool(name="dram", bufs=1, space="DRAM") as dram,
            tc.tile_pool(name="psum", bufs=2, space="PSUM") as psum,
            tc.tile_pool(name="mlp", bufs=3) as mlp,
            tc.tile_pool(name="gp", bufs=4) as gp,
            tc.tile_pool(name="sp", bufs=4) as sp,
        ):
            # one Shared AllGather output per iteration (single-writer rule)
            z_fulls = [
                dram.tile([ZROWS, F], F32, addr_space="Shared", name=f"z_full_{j}")
                for j in range(K)
            ]
            z_shard = dram.tile([SHARD, F], F32, name="z_shard")

            # ---- resident constants ----
            idx_sb = consts.tile([P, total_S], I32, name="idx_sb")
            nc.sync.dma_start(out=idx_sb[:], in_=idx_d[:])
            deg_sb = consts.tile([P, NCHUNK], F32, name="deg_sb")
            nc.sync.dma_start(out=deg_sb[:], in_=deg_d[:])

            mask = consts.tile([P, NCHUNK], F32, name="mask")
            nc.vector.tensor_scalar(out=mask[:], in0=deg_sb[:], scalar1=0.0,
                                    scalar2=None, op0=mybir.AluOpType.is_gt)
            dsq = consts.tile([P, NCHUNK], F32, name="dsq")
            nc.vector.tensor_scalar_max(out=dsq[:], in0=deg_sb[:], scalar1=1.0)
            nc.scalar.activation(out=dsq[:], in_=dsq[:],
                                 func=mybir.ActivationFunctionType.Sqrt)
            nc.vector.reciprocal(out=dsq[:], in_=dsq[:])
            nc.vector.tensor_tensor(out=dsq[:], in0=dsq[:], in1=mask[:],
                                    op=mybir.AluOpType.mult)
            dinv = consts.tile([P, NCHUNK], F32, name="dinv")
            nc.vector.tensor_tensor(out=dinv[:], in0=dsq[:], in1=dsq[:],
                                    op=mybir.AluOpType.mult)

            out_acc = consts.tile([P, NCHUNK * F], F32, name="out_acc")

            # zero row of this core's shard (gathered by padding slots)
            ztile = consts.tile([1, F], F32, name="ztile")
            nc.vector.memset(ztile[:], 0.0)
            nc.sync.dma_start(out=z_shard[NPC:NPC + 1, :], in_=ztile[:])

            # MLP weights (transposed-output layout: channels on partitions)
            w1 = []  # w1[h][k]: [128(K), 128(M=channels h*128..)]
            for h in range(D_H // P):
                w1.append([])
                for k in range(D_IN // P):
                    t = consts.tile([P, P], F32, name=f"w1_{h}_{k}")
                    nc.sync.dma_start(
                        out=t[:], in_=W1_d[k * P:(k + 1) * P, h * P:(h + 1) * P])
                    w1[h].append(t)
            w2 = []
            for k in range(D_H // P):
                t = consts.tile([P, F], F32, name=f"w2_{k}")
                nc.sync.dma_start(out=t[:], in_=W2_d[k * P:(k + 1) * P, :])
                w2.append(t)
            # biases as flat rows; applied as a K=1 matmul against a ones-row
            # (per-partition [P,1] DMAs of 4B/partition are unreliable on HW)
            b1r = []
            for h in range(D_H // P):
                t = consts.tile([1, P], F32, name=f"b1r_{h}")
                nc.sync.dma_start(out=t[:], in_=b1_d[None, h * P:(h + 1) * P])
                b1r.append(t)
            b2r = consts.tile([1, F], F32, name="b2r")
            nc.sync.dma_start(out=b2r[:], in_=b2_d[None, :])
            ones = consts.tile([1, 512], F32, name="ones")
            nc.vector.memset(ones[:], 1.0)

            ident = consts.tile([P, P], F32, name="ident")
            make_identity(nc, ident[:])

            c0 = float(cj[0])

            # ---- MLP: x^T = W2^T relu(W1^T nfT + b1) + b2, then per-128 transpose ----
            ntiles = []
            nleft = NPC
            while nleft > 0:
                t = min(512, nleft)
                ntiles.append(t)
                nleft -= t
            n0 = 0
            for NT in ntiles:
                nf = []
                for k in range(D_IN // P):
                    t = mlp.tile([P, 512], F32, tag="nf", name=f"nf_{n0}_{k}")
                    nc.sync.dma_start(
                        out=t[:, :NT], in_=nfT_d[k * P:(k + 1) * P, n0:n0 + NT])
                    nf.append(t)
                hs = []
                for h in range(D_H // P):
                    hp = psum.tile([P, 512], F32, tag="hpsum", name=f"hp_{n0}_{h}")
                    for k in range(D_IN // P):
                        nc.tensor.matmul(
                            out=hp[:, :NT], lhsT=w1[h][k][:], rhs=nf[k][:, :NT],
                            start=(k == 0), stop=False)
                    nc.tensor.matmul(
                        out=hp[:, :NT], lhsT=b1r[h][:], rhs=ones[:, :NT],
                        start=False, stop=True)
                    ht = mlp.tile([P, 512], F32, tag=f"h{h}", name=f"h_{n0}_{h}")
                    nc.scalar.activation(
                        out=ht[:, :NT], in_=hp[:, :NT],
                        func=mybir.ActivationFunctionType.Relu,
                        bias=0.0, scale=1.0)
                    hs.append(ht)
                xp = psum.tile([F, 512], F32, tag="xpsum", name=f"xp_{n0}")
                for k in range(D_H // P):
                    nc.tensor.matmul(
                        out=xp[:, :NT], lhsT=w2[k][:], rhs=hs[k][:, :NT],
                        start=(k == 0), stop=False)
                nc.tensor.matmul(
                    out=xp[:, :NT], lhsT=b2r[:], rhs=ones[:, :NT],
                    start=False, stop=True)
                xt = mlp.tile([F, 512], F32, tag="xt", name=f"xt_{n0}")
                nc.scalar.activation(
                    out=xt[:, :NT], in_=xp[:, :NT],
                    func=mybir.ActivationFunctionType.Copy,
                    bias=0.0, scale=1.0)
                for b in range(NT // P):
                    kc = n0 // P + b
                    tp = psum.tile([P, F], F32, tag="tp", name=f"tp_{kc}")
                    nc.tensor.transpose(
                        out=tp[:], in_=xt[:, b * P:(b + 1) * P], identity=ident[:F, :F])
                    nc.vector.tensor_scalar_mul(
                        out=out_acc[:, kc * F:(kc + 1) * F], in0=tp[:], scalar1=c0)
                    z0 = sp.tile([P, F], F32, tag="z0", name=f"z0_{kc}")
                    nc.vector.tensor_scalar(
                        out=z0[:], in0=tp[:], scalar1=dsq[:, kc:kc + 1],
                        scalar2=None, op0=mybir.AluOpType.mult)
                    nc.sync.dma_start(
                        out=z_shard[kc * P:(kc + 1) * P, :], in_=z0[:])
                n0 += NT

            S_max = int(max(S_k))
            rg = [list(range(NC))]

            nc.gpsimd.collective_compute(
                "AllGather", mybir.AluOpType.bypass, replica_groups=rg,
                ins=[z_shard[:].opt()], outs=[z_fulls[0][:].opt()])

            # ---- K aggregation iterations ----
            for j in range(1, K + 1):
                z_src = z_fulls[j - 1]
                cjd = sp.tile([P, NCHUNK], F32, tag="cjd", name=f"cjd_{j}")
                nc.vector.tensor_scalar_mul(out=cjd[:], in0=dsq[:],
                                            scalar1=float(cj[j]))
                for k in range(NCHUNK):
                    Sk = int(S_k[k])
                    o = int(off[k])
                    g = gp.tile([P, S_max * F], F32, tag="g", name=f"g_{j}_{k}")
                    nc.gpsimd.indirect_dma_start(
                        out=g[:, :Sk * F], out_offset=None,
                        in_=z_src[:],
                        in_offset=bass.IndirectOffsetOnAxis(
                            ap=idx_sb[:, o:o + Sk], axis=0),
                    )
                    # contiguous in-place tree reduction over the slots
                    s = Sk
                    while s > 1:
                        h = s // 2
                        nc.vector.tensor_tensor(
                            out=g[:, :h * F], in0=g[:, :h * F],
                            in1=g[:, (s - h) * F:s * F],
                            op=mybir.AluOpType.add)
                        s = s - h
                    st = g[:, :F]
                    tmp = sp.tile([P, F], F32, tag="tmp", name=f"tmp_{j}_{k}")
                    nc.vector.tensor_scalar(
                        out=tmp[:], in0=st, scalar1=cjd[:, k:k + 1],
                        scalar2=None, op0=mybir.AluOpType.mult)
                    nc.vector.tensor_tensor(
                        out=out_acc[:, k * F:(k + 1) * F],
                        in0=out_acc[:, k * F:(k + 1) * F], in1=tmp[:],
                        op=mybir.AluOpType.add)
                    if j < K:
                        zt = sp.tile([P, F], F32, tag="zt", name=f"zt_{j}_{k}")
                        nc.vector.tensor_scalar(
                            out=zt[:], in0=st, scalar1=dinv[:, k:k + 1],
                            scalar2=None, op0=mybir.AluOpType.mult)
                        nc.sync.dma_start(
                            out=z_shard[k * P:(k + 1) * P, :], in_=zt[:])
                if j < K:
                    nc.gpsimd.collective_compute(
                        "AllGather", mybir.AluOpType.bypass, replica_groups=rg,
                        ins=[z_shard[:].opt()],
                        outs=[z_fulls[j][:].opt()])

            # ---- store output: out[k*128+p, f] = out_acc[p, k*64+f] ----
            nc.sync.dma_start(
                out=out_d[:].rearrange("(k p) f -> p k f", p=P),
                in_=out_acc[:].rearrange("p (k f) -> p k f", f=F))

    nc.compile()
    return nc


_CACHE = {}


def kernel(node_feat, edge_index, W1, b1, W2, b2, temp):
    node_feat = np.asarray(node_feat, dtype=np.float32)
    edge_index = np.asarray(edge_index)
    W1 = np.ascontiguousarray(np.asarray(W1, dtype=np.float32))
    b1 = np.ascontiguousarray(np.asarray(b1, dtype=np.float32))
    W2 = np.ascontiguousarray(np.asarray(W2, dtype=np.float32))
    b2 = np.ascontiguousarray(np.asarray(b2, dtype=np.float32))
    temp = np.asarray(temp, dtype=np.float32)

    prep = _host_prep(node_feat, edge_index, temp)

    key = (edge_index.tobytes()[:4096], temp.tobytes())
    nc = _CACHE.get(key)
    if nc is None:
        nc = _build_nc(prep["S_k"], prep["off"], prep["total_S"], prep["cj"])
        _CACHE[key] = nc

    in_maps = []
    for c in range(NC):
        in_maps.append({
            "nfT": np.ascontiguousarray(prep["nfT"][c]),
            "idx": np.ascontiguousarray(prep["idx_all"][c]),
            "degpk": np.ascontiguousarray(prep["degpk"][c]),
            "W1": W1, "b1": b1, "W2": W2, "b2": b2,
        })

    res = bass_utils.run_bass_kernel_spmd(nc, in_maps, core_ids=list(range(NC)))
    global LAST_RESULTS
    LAST_RESULTS = res
    out_cat = np.concatenate([r["out"] for r in res.results], axis=0)
    return np.ascontiguousarray(out_cat[prep["pos"]])


LAST_RESULTS = None
